# revision 18
# baseline (speedup 1.0000x reference)
"""Trainium2 Bass kernel for implicit cross-attention (keys/values = queries + 1 ctx token).

Sharding: 8 cores = 4 batches x 2 head-groups (8 heads each). Each core computes
q = x_b @ Wq[:, g], causal flash-style attention over keys [ctx, q_0..q_{N-1}],
and a partial output projection out @ Wo[g, :]. Host sums the two head-group
partials per batch and adds the bias.

Attention is processed per head-pair (two heads sharing a 128-partition q^T
tile) and per 1024-query half, so both heads' score matmuls are independent
in-flight PE work while the other head's exp runs on ScalarE.
"""

import numpy as np

import concourse.bass as bass
import concourse.mybir as mybir
from concourse import bacc
from concourse.tile import TileContext
from concourse.bass_utils import run_bass_kernel_spmd
from concourse.masks import make_identity

FP = mybir.dt.float32
FPR = mybir.dt.float32r

N = 2048          # sequence length
CD = 1024         # model dim
HD = 512          # head-dim cols per core (8 heads x 64)
D = 64            # dim per head
NHEAD = 8         # heads per core
SCALE = 0.125     # D ** -0.5
NMT = HD // 128   # 4 hd chunks of 128 (2 heads each)
NCC = CD // 128   # 8 contraction chunks
NIC = N // 512    # 4 query chunks of 512
NKB = N // 128    # 16 key blocks of 128

USE_FPR = True    # float32r (full-rate PE fp32) for all matmuls
MMDT = FPR if USE_FPR else FP


def _f32(ap):
    return ap.bitcast(FP) if USE_FPR else ap


def _bc(ap):
    return ap.bitcast(FPR) if USE_FPR else ap


def _build_nc():
    nc = bacc.Bacc("TRN2", target_bir_lowering=False)
    x_d = nc.declare_dram_parameter("x", [N, CD], FP, isOutput=False)
    wq_d = nc.declare_dram_parameter("wq", [CD, HD], FP, isOutput=False)
    wk_d = nc.declare_dram_parameter("wk", [CD, HD], FP, isOutput=False)
    wv_d = nc.declare_dram_parameter("wv", [CD, HD], FP, isOutput=False)
    wo_d = nc.declare_dram_parameter("wo", [HD, CD], FP, isOutput=False)
    ctx_d = nc.declare_dram_parameter("ctx", [1, CD], FP, isOutput=False)
    y_d = nc.declare_dram_parameter("y", [N, CD], FP, isOutput=True)

    with TileContext(nc) as tc, tc.tile_pool(name="persist", bufs=1) as pp:
        # ---- persistent SBUF tensors (one slot per tag) ----
        ident = pp.tile([128, 128], FP, tag="ident", name="ident")
        # two stacked 64x64 identities (for transposes of tiles based at partition 64)
        ident2 = pp.tile([128, 64], FP, tag="ident2", name="ident2")
        ones11 = pp.tile([1, 1], FP, tag="ones11", name="ones11")
        ones16 = pp.tile([128, 16], FP, tag="ones16", name="ones16")
        zeros16 = pp.tile([128, 16], FP, tag="zeros16", name="zeros16")
        ctxT_sb = pp.tile([128, NCC], FP, tag="ctxT_sb", name="ctxT_sb")
        kctx_sb = pp.tile([1, HD], FP, tag="kctx_sb", name="kctx_sb")
        kct_sb = pp.tile([64, NHEAD], MMDT, tag="kct_sb", name="kct_sb")
        # zero-padded k_ctx^T columns per head pair (K=128 ctx score matmuls)
        kct2 = pp.tile([128, NHEAD], MMDT, tag="kct2", name="kct2")
        vctx_row = pp.tile([1, NHEAD * (D + 1)], MMDT, tag="vctx_row", name="vctx_row")
        qkT = [pp.tile([128, N], MMDT, tag=f"qkT{m}", name=f"qkT{m}") for m in range(NMT)]
        vsb = [pp.tile([128, NKB, D + 1], MMDT, tag=f"vsb{h}", name=f"vsb{h}") for h in range(NHEAD)]
        attnT = [pp.tile([128, N], MMDT, tag=f"attnT{m}", name=f"attnT{m}") for m in range(NMT)]
        wq_sb = [pp.tile([128, HD], MMDT, tag=f"wq_sb{c}", name=f"wq_sb{c}") for c in range(NCC)]
        wo_sb = [pp.tile([128, CD], MMDT, tag=f"wo_sb{m}", name=f"wo_sb{m}") for m in range(NMT)]
        xT_sb = [pp.tile([128, 512], MMDT, tag=f"xT_sb{c}", name=f"xT_sb{c}") for c in range(NCC)]

        make_identity(nc, ident)
        nc.gpsimd.memset(ident2, 0.0)
        # ident2[p, f] = 1 where p == f or p == f + 64
        nc.gpsimd.affine_select(
            out=ident2, in_=ident2, compare_op=mybir.AluOpType.not_equal,
            fill=1.0, base=0, pattern=[[-1, 64]], channel_multiplier=1)
        nc.gpsimd.affine_select(
            out=ident2, in_=ident2, compare_op=mybir.AluOpType.not_equal,
            fill=1.0, base=-64, pattern=[[-1, 64]], channel_multiplier=1)
        nc.vector.memset(ones11, 1.0)
        nc.vector.memset(ones16, 1.0)
        nc.vector.memset(zeros16, 0.0)

        # ---- weight / input DMA ----
        for c in range(NCC):
            nc.sync.dma_start(wq_sb[c], _bc(wq_d[128 * c:128 * (c + 1), :]))
        for m in range(NMT):
            nc.sync.dma_start(wo_sb[m], _bc(wo_d[128 * m:128 * (m + 1), :]))
        ctx_sb = pp.tile([1, CD], FP, tag="ctx_sb", name="ctx_sb")
        nc.sync.dma_start(ctx_sb, ctx_d[0:1, :])

        with tc.tile_pool(name="qp", bufs=2, space="PSUM") as qp_pool, \
             tc.tile_pool(name="tp", bufs=2, space="PSUM") as tp_pool, \
             tc.tile_pool(name="wkv", bufs=4) as wkv_pool, \
             tc.tile_pool(name="xn", bufs=5) as xn_pool:

            # ---- context k/v projections ----
            # ctx^T via K=1 matmuls: out[128,1] = ctx_chunk.T @ ones
            ctxT_ps = qp_pool.tile([128, 512], FP, tag="qp")
            for c in range(NCC):
                nc.tensor.matmul(ctxT_ps[:, c:c + 1], ctx_sb[0:1, 128 * c:128 * (c + 1)],
                                 ones11, start=True, stop=True)
            nc.vector.tensor_copy(ctxT_sb, ctxT_ps[:, 0:NCC])

            # k_ctx / v_ctx as natural rows: out[1, 512] = ctx_chunk.T @ W chunk
            kv_ps = qp_pool.tile([128, 1024], FP, tag="qp")
            for (w_d, base) in ((wk_d, 0), (wv_d, 512)):
                for c in range(NCC):
                    wt = wkv_pool.tile([128, HD], FP, tag="wkv")
                    nc.sync.dma_start(wt, w_d[128 * c:128 * (c + 1), :])
                    nc.tensor.matmul(kv_ps[0:1, base:base + 512],
                                     ctxT_sb[:, c:c + 1], wt,
                                     start=(c == 0), stop=(c == NCC - 1))
            nc.vector.tensor_copy(kctx_sb, kv_ps[0:1, 0:512])
            nc.vector.tensor_copy(
                vctx_row.rearrange("p (h e) -> p h e", e=D + 1)[:, :, 0:D],
                kv_ps[0:1, 512:512 + NHEAD * D].rearrange("p (h e) -> p h e", e=D))
            nc.vector.tensor_copy(
                vctx_row.rearrange("p (h e) -> p h e", e=D + 1)[:, :, D:D + 1],
                ones16[0:1, 0:NHEAD])

            # k_ctx^T per head (transpose lands at partition 0; SBUF->SBUF DMA
            # shifts odd heads to the 64-partition band of kct2)
            kct_ps = tp_pool.tile([128, 512], FP, tag="tp")
            for h in range(NHEAD):
                nc.tensor.transpose(kct_ps[0:64, h:h + 1],
                                    kctx_sb[0:1, 64 * h:64 * h + 64], ones11)
            nc.vector.tensor_copy(kct_sb, kct_ps[0:64, 0:NHEAD])
            nc.vector.tensor_copy(kct2, zeros16[:, 0:NHEAD])
            for h in range(NHEAD):
                if h % 2 == 0:
                    nc.vector.tensor_copy(kct2[0:64, h:h + 1], kct_sb[:, h:h + 1])
                else:
                    nc.sync.dma_start(kct2[64:128, h:h + 1], kct_sb[:, h:h + 1])

            # ---- x^T and q^T (per 512-query chunk) ----
            for ic in range(NIC):
                xnat = []
                for s in range(4):
                    xt = xn_pool.tile([128, CD], FP, tag="xn")
                    nc.sync.dma_start(xt, x_d[512 * ic + 128 * s: 512 * ic + 128 * (s + 1), :])
                    xnat.append(xt)
                for c in range(NCC):
                    tps = tp_pool.tile([128, 512], FP, tag="tp")
                    for s in range(4):
                        nc.tensor.transpose(tps[:, 128 * s:128 * (s + 1)],
                                            xnat[s][:, 128 * c:128 * (c + 1)], ident)
                    nc.vector.tensor_copy(xT_sb[c], tps)
                for m in range(NMT):
                    qps = qp_pool.tile([128, 512], FP, tag="qp")
                    for c in range(NCC):
                        nc.tensor.matmul(qps,
                                         wq_sb[c][:, 128 * m:128 * (m + 1)],
                                         xT_sb[c],
                                         start=(c == 0), stop=(c == NCC - 1))
                    nc.vector.tensor_copy(qkT[m][:, 512 * ic: 512 * (ic + 1)], qps)

        # ---- attention (per head pair, per 1024-query half) ----
        with tc.tile_pool(name="ps", bufs=2, space="PSUM") as ps_pool, \
             tc.tile_pool(name="pu", bufs=1, space="PSUM") as pu_pool, \
             tc.tile_pool(name="psb", bufs=4) as psb_pool, \
             tc.tile_pool(name="rc", bufs=2) as rc_pool, \
             tc.tile_pool(name="pcx", bufs=4) as pcx_pool:
            for m in range(NMT):
                heads = (2 * m, 2 * m + 1)
                bands = (0, 64)

                # v_aug: transpose q^T -> natural, packed 8 blocks per PSUM tile
                for hi in range(2):
                    h, band = heads[hi], bands[hi]
                    qh = qkT[m][band:band + 64, :]
                    for g in range(2):
                        vt_ps = ps_pool.tile([128, 1024], FP, tag="ps")
                        for j in range(8):
                            kb = 8 * g + j
                            nc.tensor.transpose(
                                vt_ps[:, 64 * j:64 * (j + 1)],
                                _f32(qh[:, 128 * kb: 128 * (kb + 1)]),
                                ident2[band:band + 64, 0:64])
                        nc.vector.tensor_copy(
                            vsb[h][:, 8 * g:8 * (g + 1), 0:D],
                            vt_ps[:, 0:512].rearrange("p (j e) -> p j e", e=D))
                    nc.vector.tensor_copy(vsb[h][:, :, D:D + 1], ones16[:, 0:NKB])

                for half in range(2):
                    q_lo, q_hi = 1024 * half, 1024 * (half + 1)
                    kb_hi = 8 * (half + 1)           # last key block for this half
                    pu = [pu_pool.tile([65, 1024], FP, tag=f"pu{hi}", name=f"pu{hi}") for hi in range(2)]

                    # ctx rows: per head, S_ctx -> [1, 1024] psum row 0 (zero-padded
                    # K=128 stationary column), exp, then K=1 outer-product into U
                    for hi in range(2):
                        h = heads[hi]
                        sc_ps = ps_pool.tile([128, 1024], FP, tag="ps")
                        for s in range(2):
                            nc.tensor.matmul(sc_ps[0:1, 512 * s:512 * (s + 1)],
                                             kct2[:, h:h + 1],
                                             qkT[m][:, q_lo + 512 * s:q_lo + 512 * (s + 1)],
                                             start=True, stop=True)
                        pcx = pcx_pool.tile([1, 1024], MMDT, tag="pcx", name="pcx")
                        nc.scalar.activation(pcx, sc_ps[0:1, :],
                                             mybir.ActivationFunctionType.Exp, scale=SCALE)
                        for s in range(2):
                            nc.tensor.matmul(pu[hi][:, 512 * s:512 * (s + 1)],
                                             vctx_row[0:1, 65 * h:65 * h + 65],
                                             pcx[0:1, 512 * s:512 * (s + 1)],
                                             start=True, stop=False)

                    # key blocks
                    for kb in range(1, kb_hi + 1):
                        i0 = 128 * (kb - 1)          # first query that sees this block
                        lo = max(i0, q_lo)
                        off = lo - q_lo
                        ptk = []
                        for hi in range(2):
                            h, band = heads[hi], bands[hi]
                            qh = qkT[m][band:band + 64, :]
                            keys = qh[:, 128 * (kb - 1): 128 * kb]
                            sp = ps_pool.tile([128, 1024], FP, tag="ps")
                            q0 = lo
                            while q0 < q_hi:
                                q1 = min(512 * (q0 // 512 + 1), q_hi)
                                o = q0 - q_lo
                                nc.tensor.matmul(sp[:, o:o + (q1 - q0)],
                                                 keys, qh[:, q0:q1],
                                                 start=True, stop=True)
                                q0 = q1
                            pt = psb_pool.tile([128, 1024], MMDT, tag="psb")
                            ptk.append(pt)
                            nc.scalar.activation(pt[:, off:1024], sp[:, off:1024],
                                                 mybir.ActivationFunctionType.Exp,
                                                 scale=SCALE)
                            if lo == i0:
                                # mask cols [i0, i0+128): keep where icol >= key row
                                nc.gpsimd.affine_select(
                                    out=pt[:, off:off + 128], in_=pt[:, off:off + 128],
                                    compare_op=mybir.AluOpType.is_ge, fill=0.0,
                                    base=0, pattern=[[1, 128]], channel_multiplier=-1)
                        for hi in range(2):
                            h = heads[hi]
                            q0 = lo
                            while q0 < q_hi:
                                q1 = min(512 * (q0 // 512 + 1), q_hi)
                                o = q0 - q_lo
                                last_kb = min(kb_hi, (q0 // 512) * 4 + 4)
                                nc.tensor.matmul(pu[hi][:, o:o + (q1 - q0)],
                                                 vsb[h][:, kb - 1, :],
                                                 ptk[hi][:, o:o + (q1 - q0)],
                                                 start=False, stop=(kb == last_kb))
                                q0 = q1

                    # normalize: attnT = U[0:64] / U[64]
                    for hi in range(2):
                        h, band = heads[hi], bands[hi]
                        for s in range(2):
                            sl_l = slice(512 * s, 512 * (s + 1))
                            sl_g = slice(q_lo + 512 * s, q_lo + 512 * (s + 1))
                            recip_sb = rc_pool.tile([1, 512], FP, tag="rcs")
                            recip_bc = rc_pool.tile([64, 512], FP, tag="rcb")
                            nc.vector.reciprocal_approx_fast(recip_sb, pu[hi][64:65, sl_l])
                            nc.gpsimd.partition_broadcast(recip_bc, recip_sb)
                            nc.vector.tensor_mul(attnT[m][band:band + 64, sl_g],
                                                 pu[hi][0:64, sl_l], recip_bc)

        # ---- output projection ----
        with tc.tile_pool(name="py", bufs=2, space="PSUM") as py_pool, \
             tc.tile_pool(name="ysb", bufs=2) as y_pool:
            for nb in range(N // 128):
                py = py_pool.tile([128, CD], FP, tag="py")
                for co in range(2):
                    for m in range(NMT):
                        nc.tensor.matmul(py[:, 512 * co:512 * (co + 1)],
                                         attnT[m][:, 128 * nb:128 * (nb + 1)],
                                         wo_sb[m][:, 512 * co:512 * (co + 1)],
                                         start=(m == 0), stop=(m == NMT - 1))
                ysb = y_pool.tile([128, CD], FP, tag="ysb")
                nc.vector.tensor_copy(ysb, py)
                nc.sync.dma_start(y_d[128 * nb:128 * (nb + 1), :], ysb)

    nc.compile()
    return nc


_NC = None


def _get_nc():
    global _NC
    if _NC is None:
        _NC = _build_nc()
    return _NC


def _shard(inputs):
    x = np.ascontiguousarray(np.asarray(inputs["x"], dtype=np.float32))
    context = np.ascontiguousarray(np.asarray(inputs["context"], dtype=np.float32))
    Wq = np.asarray(inputs["Wq"], dtype=np.float32)
    Wk = np.asarray(inputs["Wk"], dtype=np.float32)
    Wv = np.asarray(inputs["Wv"], dtype=np.float32)
    Wo = np.asarray(inputs["Wo"], dtype=np.float32)
    in_maps = []
    for c in range(8):
        b, g = c // 2, c % 2
        sl = slice(HD * g, HD * (g + 1))
        in_maps.append({
            "x": np.ascontiguousarray(x[b]),
            "wq": np.ascontiguousarray(Wq[:, sl]),
            "wk": np.ascontiguousarray(Wk[:, sl]),
            "wv": np.ascontiguousarray(Wv[:, sl]),
            "wo": np.ascontiguousarray(Wo[sl, :]),
            "ctx": np.ascontiguousarray(context[b:b + 1]),
        })
    return in_maps


def _run(inputs, trace=False, **kw):
    nc = _get_nc()
    in_maps = _shard(inputs)
    res = run_bass_kernel_spmd(nc, in_maps, list(range(8)), trace=trace, **kw)
    bo = np.asarray(inputs["bo"], dtype=np.float32)
    B = np.asarray(inputs["x"]).shape[0]
    y = np.empty((B, N, CD), dtype=np.float32)
    for b in range(B):
        y[b] = res.results[2 * b]["y"] + res.results[2 * b + 1]["y"] + bo
    return y, res


def kernel(**inputs):
    y, _ = _run(inputs)
    return y


# revision 22
# speedup vs baseline: 1.0103x; 1.0103x over previous
"""Trainium2 Bass kernel for implicit cross-attention (keys/values = queries + 1 ctx token).

Sharding: 8 cores = 4 batches x 2 head-groups (8 heads each). Each core computes
q = x_b @ Wq[:, g], causal flash-style attention over keys [ctx, q_0..q_{N-1}],
and a partial output projection out @ Wo[g, :]. Host sums the two head-group
partials per batch and adds the bias.

Attention is processed per head-pair (two heads sharing a 128-partition q^T
tile) and per 1024-query half, so both heads' score matmuls are independent
in-flight PE work while the other head's exp runs on ScalarE.
"""

import numpy as np

import concourse.bass as bass
import concourse.mybir as mybir
from concourse import bacc
from concourse.tile import TileContext
from concourse.bass_utils import run_bass_kernel_spmd
from concourse.masks import make_identity

FP = mybir.dt.float32
FPR = mybir.dt.float32r
BF = mybir.dt.bfloat16

N = 2048          # sequence length
CD = 1024         # model dim
HD = 512          # head-dim cols per core (8 heads x 64)
D = 64            # dim per head
NHEAD = 8         # heads per core
SCALE = 0.125     # D ** -0.5
NMT = HD // 128   # 4 hd chunks of 128 (2 heads each)
NCC = CD // 128   # 8 contraction chunks
NIC = N // 512    # 4 query chunks of 512
NKB = N // 128    # 16 key blocks of 128

USE_FPR = True    # float32r (full-rate PE fp32) for all matmuls
MMDT = FPR if USE_FPR else FP


def _f32(ap):
    return ap.bitcast(FP) if USE_FPR else ap


def _bc(ap):
    return ap.bitcast(FPR) if USE_FPR else ap


def _build_nc():
    nc = bacc.Bacc("TRN2", target_bir_lowering=False)
    x_d = nc.declare_dram_parameter("x", [N, CD], FP, isOutput=False)
    wq_d = nc.declare_dram_parameter("wq", [CD, HD], FP, isOutput=False)
    wk_d = nc.declare_dram_parameter("wk", [CD, HD], FP, isOutput=False)
    wv_d = nc.declare_dram_parameter("wv", [CD, HD], FP, isOutput=False)
    wo_d = nc.declare_dram_parameter("wo", [HD, CD], FP, isOutput=False)
    ctx_d = nc.declare_dram_parameter("ctx", [1, CD], FP, isOutput=False)
    y_d = nc.declare_dram_parameter("y", [N, CD], FP, isOutput=True)

    with TileContext(nc) as tc, tc.tile_pool(name="persist", bufs=1) as pp:
        # ---- persistent SBUF tensors (one slot per tag) ----
        ident = pp.tile([128, 128], FP, tag="ident", name="ident")
        # two stacked 64x64 identities (for transposes of tiles based at partition 64)
        ident2 = pp.tile([128, 64], FP, tag="ident2", name="ident2")
        ones11 = pp.tile([1, 1], FP, tag="ones11", name="ones11")
        ones16 = pp.tile([128, 16], FP, tag="ones16", name="ones16")
        zeros16 = pp.tile([128, 16], FP, tag="zeros16", name="zeros16")
        ctxT_sb = pp.tile([128, NCC], FP, tag="ctxT_sb", name="ctxT_sb")
        kctx_sb = pp.tile([1, HD], FP, tag="kctx_sb", name="kctx_sb")
        kct_sb = pp.tile([64, NHEAD], MMDT, tag="kct_sb", name="kct_sb")
        # zero-padded k_ctx^T columns per head pair (K=128 ctx score matmuls)
        kct2 = pp.tile([128, NHEAD], MMDT, tag="kct2", name="kct2")
        vctx_row = pp.tile([1, NHEAD * (D + 1)], BF, tag="vctx_row", name="vctx_row")
        qkT = [pp.tile([128, N], MMDT, tag=f"qkT{m}", name=f"qkT{m}") for m in range(NMT)]
        pcx_all = [pp.tile([1, N], BF, tag=f"pcx{h}", name=f"pcx{h}") for h in range(NHEAD)]
        vsb = [pp.tile([128, NKB, D + 1], MMDT, tag=f"vsb{h}", name=f"vsb{h}") for h in range(NHEAD)]
        attnT = [pp.tile([128, N], MMDT, tag=f"attnT{m}", name=f"attnT{m}") for m in range(NMT)]
        wq_sb = [pp.tile([128, HD], MMDT, tag=f"wq_sb{c}", name=f"wq_sb{c}") for c in range(NCC)]
        wo_sb = [pp.tile([128, CD], MMDT, tag=f"wo_sb{m}", name=f"wo_sb{m}") for m in range(NMT)]
        xT_sb = [pp.tile([128, 512], MMDT, tag=f"xT_sb{c}", name=f"xT_sb{c}") for c in range(NCC)]

        make_identity(nc, ident)
        nc.gpsimd.memset(ident2, 0.0)
        # ident2[p, f] = 1 where p == f or p == f + 64
        nc.gpsimd.affine_select(
            out=ident2, in_=ident2, compare_op=mybir.AluOpType.not_equal,
            fill=1.0, base=0, pattern=[[-1, 64]], channel_multiplier=1)
        nc.gpsimd.affine_select(
            out=ident2, in_=ident2, compare_op=mybir.AluOpType.not_equal,
            fill=1.0, base=-64, pattern=[[-1, 64]], channel_multiplier=1)
        nc.vector.memset(ones11, 1.0)
        nc.vector.memset(ones16, 1.0)
        nc.vector.memset(zeros16, 0.0)

        # ---- weight / input DMA ----
        for c in range(NCC):
            nc.sync.dma_start(wq_sb[c], _bc(wq_d[128 * c:128 * (c + 1), :]))
        for m in range(NMT):
            nc.sync.dma_start(wo_sb[m], _bc(wo_d[128 * m:128 * (m + 1), :]))
        ctx_sb = pp.tile([1, CD], FP, tag="ctx_sb", name="ctx_sb")
        nc.sync.dma_start(ctx_sb, ctx_d[0:1, :])

        with tc.tile_pool(name="qp", bufs=2, space="PSUM") as qp_pool, \
             tc.tile_pool(name="tp", bufs=2, space="PSUM") as tp_pool, \
             tc.tile_pool(name="wkv", bufs=2) as wkv_pool, \
             tc.tile_pool(name="xn", bufs=4) as xn_pool:

            # ---- context k/v projections ----
            # ctx^T via K=1 matmuls: out[128,1] = ctx_chunk.T @ ones
            ctxT_ps = qp_pool.tile([128, 512], FP, tag="qp")
            for c in range(NCC):
                nc.tensor.matmul(ctxT_ps[:, c:c + 1], ctx_sb[0:1, 128 * c:128 * (c + 1)],
                                 ones11, start=True, stop=True)
            nc.vector.tensor_copy(ctxT_sb, ctxT_ps[:, 0:NCC])

            # k_ctx / v_ctx as natural rows: out[1, 512] = ctx_chunk.T @ W chunk
            kv_ps = qp_pool.tile([128, 1024], FP, tag="qp")
            for (w_d, base) in ((wk_d, 0), (wv_d, 512)):
                for c in range(NCC):
                    wt = wkv_pool.tile([128, HD], FP, tag="wkv")
                    nc.sync.dma_start(wt, w_d[128 * c:128 * (c + 1), :])
                    nc.tensor.matmul(kv_ps[0:1, base:base + 512],
                                     ctxT_sb[:, c:c + 1], wt,
                                     start=(c == 0), stop=(c == NCC - 1))
            nc.vector.tensor_copy(kctx_sb, kv_ps[0:1, 0:512])
            nc.vector.tensor_copy(
                vctx_row.rearrange("p (h e) -> p h e", e=D + 1)[:, :, 0:D],
                kv_ps[0:1, 512:512 + NHEAD * D].rearrange("p (h e) -> p h e", e=D))
            nc.vector.tensor_copy(
                vctx_row.rearrange("p (h e) -> p h e", e=D + 1)[:, :, D:D + 1],
                ones16[0:1, 0:NHEAD])

            # k_ctx^T per head (transpose lands at partition 0; SBUF->SBUF DMA
            # shifts odd heads to the 64-partition band of kct2)
            kct_ps = tp_pool.tile([128, 512], FP, tag="tp")
            for h in range(NHEAD):
                nc.tensor.transpose(kct_ps[0:64, h:h + 1],
                                    kctx_sb[0:1, 64 * h:64 * h + 64], ones11)
            nc.vector.tensor_copy(kct_sb, kct_ps[0:64, 0:NHEAD])
            nc.vector.tensor_copy(kct2, zeros16[:, 0:NHEAD])
            for h in range(NHEAD):
                if h % 2 == 0:
                    nc.vector.tensor_copy(kct2[0:64, h:h + 1], kct_sb[:, h:h + 1])
                else:
                    nc.sync.dma_start(kct2[64:128, h:h + 1], kct_sb[:, h:h + 1])

            # ---- x^T and q^T (per 512-query chunk) ----
            for ic in range(NIC):
                xnat = []
                for s in range(4):
                    xt = xn_pool.tile([128, CD], FP, tag="xn")
                    nc.sync.dma_start(xt, x_d[512 * ic + 128 * s: 512 * ic + 128 * (s + 1), :])
                    xnat.append(xt)
                for c in range(NCC):
                    tps = tp_pool.tile([128, 512], FP, tag="tp")
                    for s in range(4):
                        nc.tensor.transpose(tps[:, 128 * s:128 * (s + 1)],
                                            xnat[s][:, 128 * c:128 * (c + 1)], ident)
                    nc.vector.tensor_copy(xT_sb[c], tps)
                for m in range(NMT):
                    qps = qp_pool.tile([128, 512], FP, tag="qp")
                    for c in range(NCC):
                        nc.tensor.matmul(qps,
                                         wq_sb[c][:, 128 * m:128 * (m + 1)],
                                         xT_sb[c],
                                         start=(c == 0), stop=(c == NCC - 1))
                    nc.vector.tensor_copy(qkT[m][:, 512 * ic: 512 * (ic + 1)], qps)

        # ---- ctx score rows for all heads (overlaps projection tail) ----
        with tc.tile_pool(name="scp", bufs=2, space="PSUM") as scp_pool:
            for h in range(NHEAD):
                m = h // 2
                sc = scp_pool.tile([1, N], FP, tag="scp", name="sc")
                for s in range(4):
                    nc.tensor.matmul(sc[0:1, 512 * s:512 * (s + 1)],
                                     kct2[:, h:h + 1],
                                     qkT[m][:, 512 * s:512 * (s + 1)],
                                     start=True, stop=True)
                nc.scalar.activation(pcx_all[h], sc,
                                     mybir.ActivationFunctionType.Exp, scale=SCALE)

        # ---- attention (per head pair, per 1024-query half) ----
        with tc.tile_pool(name="ps", bufs=2, space="PSUM") as ps_pool, \
             tc.tile_pool(name="pu", bufs=1, space="PSUM") as pu_pool, \
             tc.tile_pool(name="psb", bufs=3) as psb_pool, \
             tc.tile_pool(name="rc", bufs=2) as rc_pool:
            for m in range(NMT):
                heads = (2 * m, 2 * m + 1)
                bands = (0, 64)

                # v_aug: transpose q^T -> natural, packed 8 blocks per PSUM tile
                for hi in range(2):
                    h, band = heads[hi], bands[hi]
                    qh = qkT[m][band:band + 64, :]
                    for g in range(2):
                        vt_ps = ps_pool.tile([128, 1024], FP, tag="ps")
                        for j in range(8):
                            kb = 8 * g + j
                            nc.tensor.transpose(
                                vt_ps[:, 64 * j:64 * (j + 1)],
                                _f32(qh[:, 128 * kb: 128 * (kb + 1)]),
                                ident2[band:band + 64, 0:64])
                        nc.vector.tensor_copy(
                            vsb[h][:, 8 * g:8 * (g + 1), 0:D],
                            vt_ps[:, 0:512].rearrange("p (j e) -> p j e", e=D))
                    nc.vector.tensor_copy(vsb[h][:, :, D:D + 1], ones16[:, 0:NKB])

                for half in range(2):
                    q_lo, q_hi = 1024 * half, 1024 * (half + 1)
                    kb_hi = 8 * (half + 1)           # last key block for this half
                    pu = [pu_pool.tile([65, 1024], FP, tag=f"pu{hi}", name=f"pu{hi}") for hi in range(2)]

                    # ctx contribution: K=1 outer product seeds each U region
                    for hi in range(2):
                        h = heads[hi]
                        for s in range(2):
                            nc.tensor.matmul(pu[hi][:, 512 * s:512 * (s + 1)],
                                             vctx_row[0:1, 65 * h:65 * h + 65],
                                             pcx_all[h][0:1, q_lo + 512 * s:q_lo + 512 * (s + 1)],
                                             start=True, stop=False)

                    # key blocks
                    for kb in range(1, kb_hi + 1):
                        i0 = 128 * (kb - 1)          # first query that sees this block
                        lo = max(i0, q_lo)
                        off = lo - q_lo
                        ptk = []
                        for hi in range(2):
                            h, band = heads[hi], bands[hi]
                            qh = qkT[m][band:band + 64, :]
                            keys = qh[:, 128 * (kb - 1): 128 * kb]
                            sp = ps_pool.tile([128, 1024], FP, tag="ps")
                            q0 = lo
                            while q0 < q_hi:
                                q1 = min(512 * (q0 // 512 + 1), q_hi)
                                o = q0 - q_lo
                                nc.tensor.matmul(sp[:, o:o + (q1 - q0)],
                                                 keys, qh[:, q0:q1],
                                                 start=True, stop=True)
                                q0 = q1
                            pt = psb_pool.tile([128, 1024], MMDT, tag="psb")
                            ptk.append(pt)
                            nc.scalar.activation(pt[:, off:1024], sp[:, off:1024],
                                                 mybir.ActivationFunctionType.Exp,
                                                 scale=SCALE)
                            if lo == i0:
                                # mask cols [i0, i0+128): keep where icol >= key row
                                nc.gpsimd.affine_select(
                                    out=pt[:, off:off + 128], in_=pt[:, off:off + 128],
                                    compare_op=mybir.AluOpType.is_ge, fill=0.0,
                                    base=0, pattern=[[1, 128]], channel_multiplier=-1)
                        for hi in range(2):
                            h = heads[hi]
                            q0 = lo
                            while q0 < q_hi:
                                q1 = min(512 * (q0 // 512 + 1), q_hi)
                                o = q0 - q_lo
                                last_kb = min(kb_hi, (q0 // 512) * 4 + 4)
                                nc.tensor.matmul(pu[hi][:, o:o + (q1 - q0)],
                                                 vsb[h][:, kb - 1, :],
                                                 ptk[hi][:, o:o + (q1 - q0)],
                                                 start=False, stop=(kb == last_kb))
                                q0 = q1

                    # normalize: attnT = U[0:64] / U[64]
                    for hi in range(2):
                        h, band = heads[hi], bands[hi]
                        for s in range(2):
                            sl_l = slice(512 * s, 512 * (s + 1))
                            sl_g = slice(q_lo + 512 * s, q_lo + 512 * (s + 1))
                            recip_sb = rc_pool.tile([1, 512], FP, tag="rcs")
                            recip_bc = rc_pool.tile([64, 512], FP, tag="rcb")
                            nc.vector.reciprocal_approx_fast(recip_sb, pu[hi][64:65, sl_l])
                            nc.gpsimd.partition_broadcast(recip_bc, recip_sb)
                            nc.vector.tensor_mul(attnT[m][band:band + 64, sl_g],
                                                 pu[hi][0:64, sl_l], recip_bc)

        # ---- output projection ----
        with tc.tile_pool(name="py", bufs=2, space="PSUM") as py_pool, \
             tc.tile_pool(name="ysb", bufs=2) as y_pool:
            for nb in range(N // 128):
                py = py_pool.tile([128, CD], FP, tag="py")
                for co in range(2):
                    for m in range(NMT):
                        nc.tensor.matmul(py[:, 512 * co:512 * (co + 1)],
                                         attnT[m][:, 128 * nb:128 * (nb + 1)],
                                         wo_sb[m][:, 512 * co:512 * (co + 1)],
                                         start=(m == 0), stop=(m == NMT - 1))
                ysb = y_pool.tile([128, CD], FP, tag="ysb")
                nc.vector.tensor_copy(ysb, py)
                nc.sync.dma_start(y_d[128 * nb:128 * (nb + 1), :], ysb)

    nc.compile()
    return nc


_NC = None


def _get_nc():
    global _NC
    if _NC is None:
        _NC = _build_nc()
    return _NC


def _shard(inputs):
    x = np.ascontiguousarray(np.asarray(inputs["x"], dtype=np.float32))
    context = np.ascontiguousarray(np.asarray(inputs["context"], dtype=np.float32))
    Wq = np.asarray(inputs["Wq"], dtype=np.float32)
    Wk = np.asarray(inputs["Wk"], dtype=np.float32)
    Wv = np.asarray(inputs["Wv"], dtype=np.float32)
    Wo = np.asarray(inputs["Wo"], dtype=np.float32)
    in_maps = []
    for c in range(8):
        b, g = c // 2, c % 2
        sl = slice(HD * g, HD * (g + 1))
        in_maps.append({
            "x": np.ascontiguousarray(x[b]),
            "wq": np.ascontiguousarray(Wq[:, sl]),
            "wk": np.ascontiguousarray(Wk[:, sl]),
            "wv": np.ascontiguousarray(Wv[:, sl]),
            "wo": np.ascontiguousarray(Wo[sl, :]),
            "ctx": np.ascontiguousarray(context[b:b + 1]),
        })
    return in_maps


def _run(inputs, trace=False, **kw):
    nc = _get_nc()
    in_maps = _shard(inputs)
    res = run_bass_kernel_spmd(nc, in_maps, list(range(8)), trace=trace, **kw)
    bo = np.asarray(inputs["bo"], dtype=np.float32)
    B = np.asarray(inputs["x"]).shape[0]
    y = np.empty((B, N, CD), dtype=np.float32)
    for b in range(B):
        y[b] = res.results[2 * b]["y"] + res.results[2 * b + 1]["y"] + bo
    return y, res


def kernel(**inputs):
    y, _ = _run(inputs)
    return y
